# revision 1
# baseline (speedup 1.0000x reference)
"""Trainium2 Bass kernel for nn_ComplexMixture: weighted complex Gram matrices.

Reference (per batch b, inputs real/imag [B,T,D] f32, weight [B,T,1] f32):
    out_r[b] = sum_t w[b,t] * (r_t r_t^T + i_t i_t^T)
    out_i[b] = sum_t w[b,t] * (i_t r_t^T - r_t i_t^T)
with B=64, T=256, D=512; outputs (out_r, out_i), each [B, D, D] f32.

Strategy: pure data-parallel over 8 NeuronCores (8 batches per core).
Per core, per batch:
  - load r, i tiles [128, 2*512] (T on partitions, 2 K-tiles of 128)
  - since w >= 0: a = round(sqrt(w)*r), c = round(sqrt(w)*i)  (fp32r rounding
    fused into the weighting multiply; fp32r matmul runs at full PE rate)
  - out_r = a^T a + c^T c ; out_i = c^T a + (-a)^T c
  - 4 accumulating matmuls per PSUM tile, 8 tiles [128,512] per batch
  - evict PSUM->SBUF (split across ACT/DVE), one 1MB DMA per output per batch
"""
import numpy as np
from contextlib import ExitStack

import concourse.bacc as bacc
import concourse.tile as tile
from concourse import mybir
from concourse.bass_utils import run_bass_kernel_spmd

F32 = mybir.dt.float32
F32R = mybir.dt.float32r
BF16 = mybir.dt.bfloat16

N_CORES = 8
B_FULL = 64
BPC = B_FULL // N_CORES  # batches per core
T, D = 256, 512
KT = T // 128            # K tiles per batch
MT = D // 128            # M tiles per output row-block


def build_nc(reps: int = 1, mm_dtype=F32R):
    """Build + compile the per-core program. `reps` > 1 wraps the whole body
    in a hardware loop (for timing only; output is idempotent)."""
    nc = bacc.Bacc("TRN2", target_bir_lowering=False, debug=False)
    real = nc.dram_tensor("real", [BPC, T, D], F32, kind="ExternalInput").ap()
    imag = nc.dram_tensor("imag", [BPC, T, D], F32, kind="ExternalInput").ap()
    weight = nc.dram_tensor("weight", [BPC, T, 1], F32, kind="ExternalInput").ap()
    out_r = nc.dram_tensor("out_r", [BPC, D, D], F32, kind="ExternalOutput").ap()
    out_i = nc.dram_tensor("out_i", [BPC, D, D], F32, kind="ExternalOutput").ap()

    with tile.TileContext(nc) as tc, ExitStack() as ctx:
        wp = ctx.enter_context(tc.tile_pool(name="wp", bufs=1))
        inp = ctx.enter_context(tc.tile_pool(name="inp", bufs=3))
        wgt = ctx.enter_context(tc.tile_pool(name="wgt", bufs=2))
        outp = ctx.enter_context(tc.tile_pool(name="outp", bufs=3))
        psp = ctx.enter_context(tc.tile_pool(name="psp", bufs=8, space="PSUM"))

        def body(_iv=None):
            # all weights for the core's batches: W[p, b*KT+kt] = w[b, kt*128+p]
            W = wp.tile([128, BPC * KT], F32, tag="W")
            nc.sync.dma_start(
                W[:], weight.rearrange("b (kt p) o -> p (b kt o)", kt=KT, p=128)
            )
            SW = wp.tile([128, BPC * KT], F32, tag="SW")
            nc.scalar.activation(SW[:], W[:], mybir.ActivationFunctionType.Sqrt)
            NSW = wp.tile([128, BPC * KT], F32, tag="NSW")
            nc.vector.tensor_scalar_mul(NSW[:], SW[:], -1.0)

            for b in range(BPC):
                rt = inp.tile([128, KT * D], F32, tag="rt")
                it = inp.tile([128, KT * D], F32, tag="it")
                for kt in range(KT):
                    sl = slice(kt * D, (kt + 1) * D)
                    nc.sync.dma_start(rt[:, sl], real[b, kt * 128:(kt + 1) * 128, :])
                    nc.sync.dma_start(it[:, sl], imag[b, kt * 128:(kt + 1) * 128, :])

                a = wgt.tile([128, KT * D], mm_dtype, tag="a")   # sqrt(w)*r
                c = wgt.tile([128, KT * D], mm_dtype, tag="c")   # sqrt(w)*i
                na = wgt.tile([128, KT * D], mm_dtype, tag="na")  # -sqrt(w)*r
                for kt in range(KT):
                    sl = slice(kt * D, (kt + 1) * D)
                    ws = SW[:, b * KT + kt:b * KT + kt + 1]
                    nws = NSW[:, b * KT + kt:b * KT + kt + 1]
                    nc.vector.tensor_scalar_mul(a[:, sl], rt[:, sl], ws)
                    nc.vector.tensor_scalar_mul(c[:, sl], it[:, sl], ws)
                    nc.gpsimd.tensor_scalar_mul(na[:, sl], rt[:, sl], nws)

                or_sb = outp.tile([128, MT * D], F32, tag="or")
                oi_sb = outp.tile([128, MT * D], F32, tag="oi")
                for mi in range(MT):
                    pr = psp.tile([128, D], F32, tag="ps")
                    pi = psp.tile([128, D], F32, tag="ps")
                    for kt in range(KT):
                        m = slice(kt * D + mi * 128, kt * D + mi * 128 + 128)
                        n = slice(kt * D, (kt + 1) * D)
                        st = kt == 0
                        nc.tensor.matmul(pr[:], a[:, m], a[:, n], start=st, stop=False)
                        nc.tensor.matmul(pi[:], c[:, m], a[:, n], start=st, stop=False)
                    for kt in range(KT):
                        m = slice(kt * D + mi * 128, kt * D + mi * 128 + 128)
                        n = slice(kt * D, (kt + 1) * D)
                        sp = kt == KT - 1
                        nc.tensor.matmul(pr[:], c[:, m], c[:, n], start=False, stop=sp)
                        nc.tensor.matmul(pi[:], na[:, m], c[:, n], start=False, stop=sp)
                    osl = slice(mi * D, (mi + 1) * D)
                    if mi % 2 == 0:
                        nc.scalar.copy(or_sb[:, osl], pr[:])
                        nc.vector.tensor_copy(oi_sb[:, osl], pi[:])
                    else:
                        nc.vector.tensor_copy(or_sb[:, osl], pr[:])
                        nc.scalar.copy(oi_sb[:, osl], pi[:])
                nc.sync.dma_start(
                    out_r[b].rearrange("(mi p) c -> p mi c", mi=MT, p=128),
                    or_sb[:].rearrange("p (mi c) -> p mi c", mi=MT, c=D),
                )
                nc.sync.dma_start(
                    out_i[b].rearrange("(mi p) c -> p mi c", mi=MT, p=128),
                    oi_sb[:].rearrange("p (mi c) -> p mi c", mi=MT, c=D),
                )

        if reps == 1:
            body()
        else:
            with tc.For_i(0, reps, 1) as iv:
                body(iv)

    nc.compile()
    return nc


_NC_CACHE = {}


def _get_nc(reps: int = 1):
    if reps not in _NC_CACHE:
        _NC_CACHE[reps] = build_nc(reps=reps)
    return _NC_CACHE[reps]


def kernel(real, imag, weight):
    real = np.ascontiguousarray(np.asarray(real, dtype=np.float32))
    imag = np.ascontiguousarray(np.asarray(imag, dtype=np.float32))
    weight = np.ascontiguousarray(np.asarray(weight, dtype=np.float32))
    assert real.shape == (B_FULL, T, D) and weight.shape == (B_FULL, T, 1)

    nc = _get_nc()
    in_maps = [
        {
            "real": real[i * BPC:(i + 1) * BPC],
            "imag": imag[i * BPC:(i + 1) * BPC],
            "weight": weight[i * BPC:(i + 1) * BPC],
        }
        for i in range(N_CORES)
    ]
    res = run_bass_kernel_spmd(nc, in_maps, list(range(N_CORES)))
    out_r = np.concatenate([res.results[i]["out_r"] for i in range(N_CORES)], axis=0)
    out_i = np.concatenate([res.results[i]["out_i"] for i in range(N_CORES)], axis=0)
    return (out_r, out_i)
